# revision 14
# baseline (speedup 1.0000x reference)
"""Bass/Trainium2 kernel for nn_HE_FM (factorization machine embedding lookup).

Computation: out[n] = W[uid[n]] + W[iid[n]+USER_NUM] + b + dot(V[uid[n]], V[iid[n]+USER_NUM])

Strategy (data parallel over batch, tables replicated on all 8 cores):
  - Host builds an augmented table A [1.5M, 66] f32:
      user rows  (r < 1M):  A[r] = [V[r], W[r], 1.0]
      item rows  (r >= 1M): A[r] = [V[r], 1.0, W[r]+b]
    so dot(A[uid], A[iid+1M]) over 66 elements equals the full FM output.
  - Each core gathers 2*2048 rows of A with indirect (SWDGE) DMA,
    multiplies elementwise and does a segmented reduce of width 66.
"""

from contextlib import ExitStack

import numpy as np

import concourse.bass as bass
import concourse.mybir as mybir
from concourse.bass_utils import run_bass_kernel_spmd

USER_NUM = 1_000_000
ITEM_NUM = 500_000
TOTAL_ROWS = USER_NUM + ITEM_NUM
D = 64
WIDTH = D + 2  # V row + [W, 1] / [1, W+b]
BATCH = 16384
N_CORES = 8
B_CORE = BATCH // N_CORES  # 2048
P = 128


def build_program(total_rows=TOTAL_ROWS, user_num=USER_NUM, b_core=B_CORE):
    """Per-core SPMD program. Parameterized so tests can build a small variant."""
    k = b_core // P  # index columns per partition
    nc = bass.Bass()
    ids = nc.declare_dram_parameter("ids", [2, b_core], mybir.dt.int32, isOutput=False)
    table = nc.declare_dram_parameter(
        "table", [total_rows, WIDTH], mybir.dt.float32, isOutput=False
    )
    outp = nc.declare_dram_parameter("out", [b_core], mybir.dt.float32, isOutput=True)

    with (
        ExitStack() as ctx,
        nc.sbuf_tensor([P, k], mybir.dt.int32) as idx_u,
        nc.sbuf_tensor([P, k], mybir.dt.int32) as idx_i,
        nc.sbuf_tensor([P, k * WIDTH], mybir.dt.float32) as au,
        nc.sbuf_tensor([P, k * WIDTH], mybir.dt.float32) as ai,
        nc.sbuf_tensor([P, k * WIDTH], mybir.dt.float32) as prod,
        nc.sbuf_tensor([P, k], mybir.dt.float32) as resv,
        nc.Block() as block,
        nc.semaphore("iu_sem") as iu_sem,
        nc.semaphore("ii_sem") as ii_sem,
        nc.semaphore("o_sem") as o_sem,
        nc.semaphore("tt_sem") as tt_sem,
        nc.semaphore("v_sem") as v_sem,
    ):
        # One semaphore per column pair: DMA-completion increments from
        # different instructions interleave across the 16 SDMA engines, so
        # only a semaphore's full value is a sound wait point.
        g_sems = [ctx.enter_context(nc.semaphore(f"g_sem{j}")) for j in range(k)]

        @block.sync
        def _(sync: bass.BassEngine):
            sync.dma_start(
                out=idx_u[:], in_=ids[0].rearrange("(p k) -> p k", p=P)
            ).then_inc(iu_sem, 16)
            sync.dma_start(
                out=idx_i[:], in_=ids[1].rearrange("(p k) -> p k", p=P)
            ).then_inc(ii_sem, 16)
            sync.wait_ge(v_sem, k)
            sync.dma_start(
                out=outp[:].rearrange("(p k) -> p k", p=P), in_=resv[:]
            ).then_inc(o_sem, 16)
            sync.wait_ge(o_sem, 16)

        @block.gpsimd
        def _(gpsimd: bass.BassEngine):
            # HW indirect DMA: one descriptor per partition per instruction,
            # using idx[p, 0] — so one gather instruction per index column.
            # Interleave u/i columns so DVE can start on column j as soon as
            # its pair of gathers lands. u-gathers only need idx_u, so the
            # first gather starts as soon as that 8KB DMA completes.
            gpsimd.wait_ge(iu_sem, 16)
            gpsimd.indirect_dma_start(
                out=au[:, 0:WIDTH],
                out_offset=None,
                in_=table[:],
                in_offset=bass.IndirectOffsetOnAxis(ap=idx_u[:, 0:1], axis=0),
            ).then_inc(g_sems[0], 16)
            gpsimd.wait_ge(ii_sem, 16)
            gpsimd.indirect_dma_start(
                out=ai[:, 0:WIDTH],
                out_offset=None,
                in_=table[:],
                in_offset=bass.IndirectOffsetOnAxis(ap=idx_i[:, 0:1], axis=0),
                element_offset=user_num * WIDTH,
            ).then_inc(g_sems[0], 16)
            for j in range(1, k):
                gpsimd.indirect_dma_start(
                    out=au[:, j * WIDTH : (j + 1) * WIDTH],
                    out_offset=None,
                    in_=table[:],
                    in_offset=bass.IndirectOffsetOnAxis(ap=idx_u[:, j : j + 1], axis=0),
                ).then_inc(g_sems[j], 16)
                gpsimd.indirect_dma_start(
                    out=ai[:, j * WIDTH : (j + 1) * WIDTH],
                    out_offset=None,
                    in_=table[:],
                    in_offset=bass.IndirectOffsetOnAxis(ap=idx_i[:, j : j + 1], axis=0),
                    element_offset=user_num * WIDTH,
                ).then_inc(g_sems[j], 16)

        @block.vector
        def _(vector: bass.BassEngine):
            for j in range(k):
                vector.wait_ge(g_sems[j], 32)
                vector.tensor_tensor(
                    out=prod[:, j * WIDTH : (j + 1) * WIDTH],
                    in0=au[:, j * WIDTH : (j + 1) * WIDTH],
                    in1=ai[:, j * WIDTH : (j + 1) * WIDTH],
                    op=mybir.AluOpType.mult,
                ).then_inc(tt_sem, 1)
                vector.wait_ge(tt_sem, j + 1)
                vector.tensor_reduce(
                    out=resv[:, j : j + 1],
                    in_=prod[:, j * WIDTH : (j + 1) * WIDTH].rearrange(
                        "p (k w) -> p k w", w=WIDTH
                    ),
                    axis=mybir.AxisListType.X,
                    op=mybir.AluOpType.add,
                ).then_inc(v_sem, 1)

    return nc


def build_table(W, b, V, total_rows=TOTAL_ROWS, user_num=USER_NUM):
    A = np.empty((total_rows, WIDTH), dtype=np.float32)
    A[:, :D] = V
    A[:user_num, D] = W[:user_num, 0]
    A[:user_num, D + 1] = 1.0
    A[user_num:, D] = 1.0
    A[user_num:, D + 1] = W[user_num:, 0] + b[0]
    return A


_program_cache = {}


def kernel(INPUT, W, b, V):
    INPUT = np.asarray(INPUT, dtype=np.int32)
    W = np.asarray(W, dtype=np.float32)
    b = np.asarray(b, dtype=np.float32)
    V = np.asarray(V, dtype=np.float32)

    if "nc" not in _program_cache:
        _program_cache["nc"] = build_program()
    nc = _program_cache["nc"]

    A = build_table(W, b, V)
    # ids[i] : [2, B_CORE] int32 — row 0 = uid, row 1 = raw iid (+USER_NUM on device)
    ids = np.ascontiguousarray(
        INPUT.reshape(N_CORES, B_CORE, 2).transpose(0, 2, 1)
    ).astype(np.int32)

    in_maps = [{"ids": ids[i], "table": A} for i in range(N_CORES)]
    res = run_bass_kernel_spmd(nc, in_maps, core_ids=list(range(N_CORES)))
    global last_results
    last_results = res
    out = np.concatenate([np.asarray(res.results[i]["out"]) for i in range(N_CORES)])
    return out.reshape(BATCH, 1).astype(np.float32)


last_results = None


# revision 15
# speedup vs baseline: 1.0061x; 1.0061x over previous
"""Bass/Trainium2 kernel for nn_HE_FM (factorization machine embedding lookup).

Computation: out[n] = W[uid[n]] + W[iid[n]+USER_NUM] + b + dot(V[uid[n]], V[iid[n]+USER_NUM])

Strategy (data parallel over batch, tables replicated on all 8 cores):
  - Host builds an augmented table A [1.5M, 66] f32:
      user rows  (r < 1M):  A[r] = [V[r], W[r], 1.0]
      item rows  (r >= 1M): A[r] = [V[r], 1.0, W[r]+b]
    so dot(A[uid], A[iid+1M]) over 66 elements equals the full FM output.
  - Each core gathers 2*2048 rows of A with indirect (SWDGE) DMA,
    multiplies elementwise and does a segmented reduce of width 66.
"""

from contextlib import ExitStack

import numpy as np

import concourse.bass as bass
import concourse.mybir as mybir
from concourse.bass_utils import run_bass_kernel_spmd

USER_NUM = 1_000_000
ITEM_NUM = 500_000
TOTAL_ROWS = USER_NUM + ITEM_NUM
D = 64
WIDTH = D + 2  # V row + [W, 1] / [1, W+b]
BATCH = 16384
N_CORES = 8
B_CORE = BATCH // N_CORES  # 2048
P = 128


def build_program(total_rows=TOTAL_ROWS, user_num=USER_NUM, b_core=B_CORE):
    """Per-core SPMD program. Parameterized so tests can build a small variant."""
    k = b_core // P  # index columns per partition
    nc = bass.Bass()
    ids = nc.declare_dram_parameter("ids", [2, b_core], mybir.dt.int32, isOutput=False)
    table = nc.declare_dram_parameter(
        "table", [total_rows, WIDTH], mybir.dt.float32, isOutput=False
    )
    outp = nc.declare_dram_parameter("out", [b_core], mybir.dt.float32, isOutput=True)

    with (
        ExitStack() as ctx,
        nc.sbuf_tensor([P, k], mybir.dt.int32) as idx_u,
        nc.sbuf_tensor([P, k], mybir.dt.int32) as idx_i,
        nc.sbuf_tensor([P, k * WIDTH], mybir.dt.float32) as au,
        nc.sbuf_tensor([P, k * WIDTH], mybir.dt.float32) as ai,
        nc.sbuf_tensor([P, k * WIDTH], mybir.dt.float32) as prod,
        nc.sbuf_tensor([P, k], mybir.dt.float32) as resv,
        nc.Block() as block,
        nc.semaphore("iu_sem") as iu_sem,
        nc.semaphore("ii_sem") as ii_sem,
        nc.semaphore("o_sem") as o_sem,
        nc.semaphore("tt_sem") as tt_sem,
        nc.semaphore("v_sem") as v_sem,
    ):
        # One semaphore per column pair: DMA-completion increments from
        # different instructions interleave across the 16 SDMA engines, so
        # only a semaphore's full value is a sound wait point.
        g_sems = [ctx.enter_context(nc.semaphore(f"g_sem{j}")) for j in range(k)]

        @block.sync
        def _(sync: bass.BassEngine):
            sync.dma_start(
                out=idx_u[:], in_=ids[0].rearrange("(p k) -> p k", p=P)
            ).then_inc(iu_sem, 16)
            sync.dma_start(
                out=idx_i[:], in_=ids[1].rearrange("(p k) -> p k", p=P)
            ).then_inc(ii_sem, 16)
            sync.wait_ge(v_sem, k)
            sync.dma_start(
                out=outp[:].rearrange("(p k) -> p k", p=P), in_=resv[:]
            ).then_inc(o_sem, 16)
            sync.wait_ge(o_sem, 16)

        @block.gpsimd
        def _(gpsimd: bass.BassEngine):
            # HW indirect DMA: one descriptor per partition per instruction,
            # using idx[p, 0] — so one gather instruction per index column.
            # Interleave u/i columns so DVE can start on column j as soon as
            # its pair of gathers lands. u-gathers only need idx_u, so the
            # first gather starts as soon as that 8KB DMA completes.
            gpsimd.wait_ge(iu_sem, 16)
            gpsimd.indirect_dma_start(
                out=au[:, 0:WIDTH],
                out_offset=None,
                in_=table[:],
                in_offset=bass.IndirectOffsetOnAxis(ap=idx_u[:, 0:1], axis=0),
            ).then_inc(g_sems[0], 16)
            gpsimd.wait_ge(ii_sem, 16)
            gpsimd.indirect_dma_start(
                out=ai[:, 0:WIDTH],
                out_offset=None,
                in_=table[:],
                in_offset=bass.IndirectOffsetOnAxis(ap=idx_i[:, 0:1], axis=0),
                element_offset=user_num * WIDTH,
            ).then_inc(g_sems[0], 16)
            for j in range(1, k):
                gpsimd.indirect_dma_start(
                    out=au[:, j * WIDTH : (j + 1) * WIDTH],
                    out_offset=None,
                    in_=table[:],
                    in_offset=bass.IndirectOffsetOnAxis(ap=idx_u[:, j : j + 1], axis=0),
                ).then_inc(g_sems[j], 16)
                gpsimd.indirect_dma_start(
                    out=ai[:, j * WIDTH : (j + 1) * WIDTH],
                    out_offset=None,
                    in_=table[:],
                    in_offset=bass.IndirectOffsetOnAxis(ap=idx_i[:, j : j + 1], axis=0),
                    element_offset=user_num * WIDTH,
                ).then_inc(g_sems[j], 16)

        @block.vector
        def _(vector: bass.BassEngine):
            for j in range(k):
                vector.wait_ge(g_sems[j], 32)
                vector.tensor_tensor(
                    out=prod[:, j * WIDTH : (j + 1) * WIDTH],
                    in0=au[:, j * WIDTH : (j + 1) * WIDTH],
                    in1=ai[:, j * WIDTH : (j + 1) * WIDTH],
                    op=mybir.AluOpType.mult,
                ).then_inc(tt_sem, 1)
                vector.wait_ge(tt_sem, j + 1)
                vector.tensor_reduce(
                    out=resv[:, j : j + 1],
                    in_=prod[:, j * WIDTH : (j + 1) * WIDTH].rearrange(
                        "p (k w) -> p k w", w=WIDTH
                    ),
                    axis=mybir.AxisListType.X,
                    op=mybir.AluOpType.add,
                ).then_inc(v_sem, 1)

    _strip_dead_const_memsets(nc)
    return nc


def _strip_dead_const_memsets(nc):
    """Bass.__init__ unconditionally memsets four const-* SBUF tensors on
    gpsimd; this kernel never reads them (birverifier agrees: "no reader"),
    and they sit on the Pool critical path ahead of the gathers."""
    for bb in nc.m.functions[0].blocks:
        keep = []
        for inst in bb.instructions:
            is_dead_const = type(inst).__name__ == "InstMemset" and any(
                getattr(out, "memref", "").startswith("const-") for out in inst.outs
            )
            if not is_dead_const:
                keep.append(inst)
        if len(keep) != len(bb.instructions):
            bb.instructions[:] = keep


def build_table(W, b, V, total_rows=TOTAL_ROWS, user_num=USER_NUM):
    A = np.empty((total_rows, WIDTH), dtype=np.float32)
    A[:, :D] = V
    A[:user_num, D] = W[:user_num, 0]
    A[:user_num, D + 1] = 1.0
    A[user_num:, D] = 1.0
    A[user_num:, D + 1] = W[user_num:, 0] + b[0]
    return A


_program_cache = {}


def kernel(INPUT, W, b, V):
    INPUT = np.asarray(INPUT, dtype=np.int32)
    W = np.asarray(W, dtype=np.float32)
    b = np.asarray(b, dtype=np.float32)
    V = np.asarray(V, dtype=np.float32)

    if "nc" not in _program_cache:
        _program_cache["nc"] = build_program()
    nc = _program_cache["nc"]

    A = build_table(W, b, V)
    # ids[i] : [2, B_CORE] int32 — row 0 = uid, row 1 = raw iid (+USER_NUM on device)
    ids = np.ascontiguousarray(
        INPUT.reshape(N_CORES, B_CORE, 2).transpose(0, 2, 1)
    ).astype(np.int32)

    in_maps = [{"ids": ids[i], "table": A} for i in range(N_CORES)]
    res = run_bass_kernel_spmd(nc, in_maps, core_ids=list(range(N_CORES)))
    global last_results
    last_results = res
    out = np.concatenate([np.asarray(res.results[i]["out"]) for i in range(N_CORES)])
    return out.reshape(BATCH, 1).astype(np.float32)


last_results = None
